# revision 6
# baseline (speedup 1.0000x reference)
"""v3 Trainium2 Bass kernel (8-core SPMD; core c owns neurons
[c*16, (c+1)*16) of every topo batch).

Structure per topo batch b:
  - v arrives as the DRAM AllGather of the previous batch's per-core
    affine outputs (the 15us collective constant dominates each
    boundary); adaptive tanh-gelu is applied to the gathered column
    (Exp and Tanh share one ACT table set - no per-batch table loads).
  - topo-norm stats via two 1-col matmuls + fast-inverse-sqrt Newton
    (2 iters) on [1,1] scalars (same-engine DVE chains pipeline at
    ~0 cost in the model); rstd / mu*rstd broadcast to per-partition
    columns with one ones-row matmul.
  - topo attention: q broadcast built as Ub = ones_mat * q (one DVE
    tensor_scalar) followed by a single bf16 matmul against the
    identity; exp reads raw PSUM with per-partition scale = k*rs.
  - neuron attention: q/k/v computed as PSUM columns (stationary =
    bf16 weights, moving = 1-col u', ~1ns each); q+bias transposed
    once ([S,16] -> [16,S], one bf16 ident matmul); per-neuron score
    tiles built with precomputed one-hot SELECTOR matmuls (stationary
    base-partition constraint forbids using qT rows directly).  The
    k*rs scale is folded into exp: 8 neurons via ACT scale-exp
    straight from PSUM, 8 via DVE tensor_scalar + two 512-col bank
    exps - balances ACT (8*292+2*612 ns) against DVE (8*258 ns).
  - PSUM scores split into 4 bank tiles so each exp only waits on its
    own bank's selector matmuls (dependency granularity).
  - PV: stationary = E (bf16), moving = interleaved [v*m | m] pairs;
    mask semantics: denominator sums exp only over unmasked keys,
    masked-query rows are killed by w*mask in the affine.
  - rs folded host-side into K weights/biases (topo and neuron),
    gamma into topo weights, v-bias*mask into a host column.
  - All weights DMA'd in bf16 (halves HBM traffic, 4x fewer PE cycles
    per moving column; rel err ~5.5e-3 vs the 2e-2 gate).
  - Weight prefetch DMAs ride the Pool (SWDGE) queue: a DMA holds its
    queue's sequencer for the whole transfer in the cost model, and
    Pool only runs the boundary collectives; ACT (exps), DVE (chain
    ops) and SP (gather in/out hops) must stay unblocked.  Batch 0's
    tqkv is split ACT+Pool to shorten the cold start.
"""
import sys
import numpy as np
import ml_dtypes

sys.path.insert(0, "/opt/trn_rl_repo")

I, L, T, S = 128, 8, 128, 128
N_CORES = 8
TL = T // N_CORES
EPS = 1e-5
RS = float(1.0 / np.sqrt(np.float32(S)))
GC = 0.7978845608028654
GA = 0.044715
MAGIC = 0x5F3759DF

_cached = None

# small column layout
C_QB = 0          # 0:16   q bias cols
C_KB = 16         # 16:32  k bias cols (x rs)
C_MT = 32         # 32:48  mask cols
C_BVMT = 48       # 48:64  v-bias*mask cols
C_WMT = 64        # 64:80  (W*mask) cols
C_TC = 80         # 80:83  topo_c (k row x rs)
C_TBP = 83        # 83:86  topo_bp
C_GAM = 86
C_BET = 87
C_G0 = 88         # ada[b-1,:,0]
C_G1H = 89        # 0.5*ada[b-1,:,1]
NC_SM = 90


def _build():
    from concourse import bacc, tile, mybir

    fp32 = mybir.dt.float32
    bf16 = mybir.dt.bfloat16
    int32 = mybir.dt.int32
    Exp = mybir.ActivationFunctionType.Exp
    Tanh = mybir.ActivationFunctionType.Tanh
    mul_op = mybir.AluOpType.mult
    add_op = mybir.AluOpType.add
    sub_op = mybir.AluOpType.subtract
    shr_op = mybir.AluOpType.arith_shift_right
    bypass = mybir.AluOpType.bypass
    div_op = mybir.AluOpType.divide

    nc = bacc.Bacc("TRN2", target_bir_lowering=False, debug=False,
                   enable_asserts=True, num_devices=N_CORES)

    tqkv_d = nc.dram_tensor("tqkv", [L, S, TL * 3 * S], bf16,
                            kind="ExternalInput").ap()
    topo_wt_d = nc.dram_tensor("topo_wt", [L, S, 3 * S], bf16,
                               kind="ExternalInput").ap()
    small_d = nc.dram_tensor("small", [L, S, NC_SM], fp32,
                             kind="ExternalInput").ap()
    pre_d = nc.dram_tensor("pre", [S, 1], fp32, kind="ExternalInput").ap()
    wbc_d = nc.dram_tensor("wbc", [TL, L], fp32, kind="ExternalInput").ap()
    ident_d = nc.dram_tensor("ident", [S, S], bf16, kind="ExternalInput").ap()
    selb_d = nc.dram_tensor("selb", [TL, TL * S], bf16, kind="ExternalInput").ap()
    magic_d = nc.dram_tensor("magic", [1, 2], int32, kind="ExternalInput").ap()
    out_d = nc.dram_tensor("out", [TL, 1], fp32, kind="ExternalOutput").ap()

    with tile.TileContext(nc) as tc:
        with tc.tile_pool(name="wpool", bufs=3) as wpool, \
             tc.tile_pool(name="spool", bufs=3) as spool, \
             tc.tile_pool(name="fixed", bufs=1) as fixed, \
             tc.tile_pool(name="work", bufs=1) as work, \
             tc.tile_pool(name="ps_big", bufs=1, space="PSUM") as ps_big, \
             tc.tile_pool(name="ps_sc", bufs=2, space="PSUM") as ps_sc:

            identb = fixed.tile([S, S], bf16)
            nc.scalar.dma_start(identb[:], ident_d)
            selb = fixed.tile([TL, TL * S], bf16)
            nc.scalar.dma_start(selb[:], selb_d)
            pre = fixed.tile([S, 1], fp32)
            nc.scalar.dma_start(pre[:], pre_d)
            wbc = fixed.tile([TL, L], fp32)
            nc.scalar.dma_start(wbc[:], wbc_d)
            magic = fixed.tile([1, 2], int32)
            nc.scalar.dma_start(magic[:], magic_d)

            ones_col = fixed.tile([S, 1], fp32)
            nc.vector.memset(ones_col[:], 1.0)
            ones_colb = fixed.tile([S, 1], bf16)
            nc.vector.memset(ones_colb[:], 1.0)
            ones_row = fixed.tile([1, S], fp32)
            nc.vector.memset(ones_row[:], 1.0)
            ones_rowb = fixed.tile([1, S], bf16)
            nc.vector.memset(ones_rowb[:], 1.0)
            ones_matb = fixed.tile([S, S], bf16)
            nc.vector.memset(ones_matb[:], 1.0)

            v_bf = work.tile([S, 1], bf16)
            u_col = work.tile([S, 1], fp32)
            sc = work.tile([1, 8], fp32)
            sci = sc[:].bitcast(int32)
            yA = work.tile([1, 1], fp32)
            yB = work.tile([1, 1], fp32)
            yAi = yA[:].bitcast(int32)
            yBi = yB[:].bitcast(int32)
            bc_sb = work.tile([S, 2], fp32)
            qkvt = work.tile([S, 3], fp32)
            qb_t = work.tile([S, 1], bf16)
            qrow_t = work.tile([1, S], bf16)
            Et_sb = work.tile([S, S], bf16)
            pvr_t = work.tile([S, 2], bf16)
            nc.vector.memset(pvr_t[:], 1.0)
            rd1 = work.tile([S, 1], fp32)
            up_f = work.tile([S, 1], fp32)
            up_bf = work.tile([S, 1], bf16)
            qkb = work.tile([S, TL], bf16)
            krs_f = work.tile([S, TL], fp32)
            qT = work.tile([TL, S], bf16)
            sc_sb = work.tile([S, TL * S], fp32)
            vm1 = work.tile([S, TL], fp32)
            pvr = work.tile([S, 2 * TL], bf16)
            pvr2 = pvr[:].rearrange("p (t k) -> p t k", k=2)
            E_sb = work.tile([S, TL * S], bf16)
            rden = work.tile([S, TL], fp32)
            zp = work.tile([S, TL], fp32)
            zpu = work.tile([S, TL], fp32)
            afr = work.tile([S, TL], fp32)
            aff_sb = work.tile([TL, 1], fp32)

            sb_t0 = ps_big.tile([S, S], fp32, name="sb_t0")
            sb_t1 = ps_big.tile([S, S], fp32, name="sb_t1")
            sbank = [ps_big.tile([S, 512], fp32, name=f"sbank{i}")
                     for i in range(4)]

            def sb_slice(tl):
                if tl == 0:
                    return sb_t0[:]
                if tl == 1:
                    return sb_t1[:]
                k = tl - 2
                return sbank[k // 4][:, (k % 4) * S:(k % 4 + 1) * S]

            ag_in = []
            ag_out = []
            for b in range(L - 1):
                ag_in.append(tc.tile([TL, 1], fp32, space="DRAM",
                                     name=f"agin{b}")[0])
                ag_out.append(tc.tile([S, 1], fp32, space="DRAM",
                                      addr_space="Shared", name=f"agout{b}")[0])

            def ts(out, in0, s1, op0, s2=None, op1=None):
                if s2 is None:
                    nc.vector.tensor_scalar(out, in0, s1, None, op0)
                else:
                    nc.vector.tensor_scalar(out, in0, s1, s2, op0, op1)

            def load_weights(b, eng, split=False):
                tq = wpool.tile([S, TL * 3 * S], bf16, tag="tq")
                if split:
                    HC = TL * 3 * S // 2
                    nc.scalar.dma_start(tq[:, 0:HC], tqkv_d[b][:, 0:HC])
                    eng.dma_start(tq[:, HC:2 * HC], tqkv_d[b][:, HC:2 * HC])
                else:
                    eng.dma_start(tq[:], tqkv_d[b])
                tw = spool.tile([S, 3 * S], bf16, tag="tw")
                eng.dma_start(tw[:], topo_wt_d[b])
                sm = spool.tile([S, NC_SM], fp32, tag="sm")
                eng.dma_start(sm[:], small_d[b])
                return tq, tw, sm

            wtiles = load_weights(0, nc.gpsimd, split=True)
            for b in range(L):
                tq, tw, sm = wtiles

                scr = ps_sc.tile([S, 512], fp32, tag="scr")
                qkv_ps = scr[:, 0:48]
                pvn_ps = scr[:, 48:80]
                aff_ps = scr[0:TL, 80:81]
                sv_ps = scr[0:1, 96:97]
                svv_ps = scr[0:1, 97:98]
                bc_ps = scr[:, 100:102]
                A_ps = scr[:, 104:107]
                pvt_ps = scr[:, 110:112]
                tr_ps = scr[0:TL, 128:256]
                trt_ps = scr[0:1, 256:384]
                tsc_ps = scr[:, 384:512]

                gam = sm[:, C_GAM:C_GAM + 1]
                bet = sm[:, C_BET:C_BET + 1]

                # ---- acquire v (gelu of previous batch's outputs) ----
                if b == 0:
                    nc.vector.tensor_copy(v_bf[:], pre[:, 0:1])
                else:
                    vin = work.tile([S, 1], fp32, tag="vin")
                    nc.sync.dma_start(vin[:], ag_out[b - 1][:])
                    g0 = sm[:, C_G0:C_G0 + 1]
                    g1h = sm[:, C_G1H:C_G1H + 1]
                    xg = work.tile([S, 1], fp32, tag="xg")
                    t1 = work.tile([S, 1], fp32, tag="t1")
                    t2 = work.tile([S, 1], fp32, tag="t2")
                    wg = work.tile([S, 1], fp32, tag="wg")
                    ts(xg[:], vin[:], g0, mul_op)
                    nc.vector.tensor_mul(t1[:], xg[:], xg[:])
                    ts(t1[:], t1[:], GA, mul_op, 1.0, add_op)
                    nc.vector.tensor_mul(t2[:], t1[:], xg[:])
                    nc.scalar.activation(t2[:], t2[:], Tanh, scale=GC)
                    ts(wg[:], xg[:], g1h, mul_op)
                    nc.vector.scalar_tensor_tensor(v_bf[:], t2[:], 1.0, wg[:],
                                                   add_op, mul_op)

                # ---- stats + Newton rsqrt (PE + DVE) ----
                nc.tensor.matmul(sv_ps, ones_colb[:], v_bf[:],
                                 start=True, stop=True)
                nc.tensor.matmul(svv_ps, v_bf[:], v_bf[:],
                                 start=True, stop=True)
                ts(sc[:, 0:1], sv_ps, 1.0 / S, mul_op)
                ts(sc[:, 1:2], svv_ps, 1.0 / S, mul_op)
                nc.vector.scalar_tensor_tensor(sc[:, 3:4], sc[:, 0:1],
                                               sc[:, 0:1], sc[:, 1:2],
                                               mul_op, sub_op)
                ts(sc[:, 4:5], sc[:, 3:4], -1.0, mul_op, EPS, add_op)
                ts(sc[:, 5:6], sc[:, 3:4], -0.5, mul_op, 0.5 * EPS, add_op)
                ts(yBi, sci[:, 4:5], 1, shr_op)
                nc.vector.tensor_sub(yAi, magic[:, 0:1], yBi)
                for _ in range(2):
                    nc.vector.scalar_tensor_tensor(yB[:], yA[:], sc[:, 5:6],
                                                   yA[:], mul_op, mul_op)
                    ts(yB[:], yB[:], -1.0, mul_op, 1.5, add_op)
                    nc.vector.tensor_mul(yA[:], yA[:], yB[:])
                nc.vector.tensor_copy(sc[:, 6:7], yA[:])
                nc.vector.tensor_mul(sc[:, 7:8], yA[:], sc[:, 0:1])
                nc.tensor.matmul(bc_ps, ones_row[:], sc[:, 6:8],
                                 start=True, stop=True)
                rstd_c = bc_ps[:, 0:1]
                murstd_c = bc_ps[:, 1:2]

                # ---- u = rstd*gamma*(v-mu) + beta ----
                gv = work.tile([S, 1], fp32, tag="gv")
                gm2 = work.tile([S, 1], fp32, tag="gm2")
                ts(gv[:], v_bf[:], gam, mul_op)
                ts(gm2[:], gam, murstd_c, mul_op, bet, sub_op)
                nc.vector.scalar_tensor_tensor(u_col[:], gv[:], rstd_c,
                                               gm2[:], mul_op, sub_op)

                # ---- topo qkv (gamma, rs folded host-side) ----
                for m in range(3):
                    nc.tensor.matmul(A_ps[:, m:m + 1], tw[:, m * S:(m + 1) * S],
                                     v_bf[:], start=True, stop=True)
                cm2 = work.tile([S, 3], fp32, tag="cm2")
                nc.vector.scalar_tensor_tensor(cm2[:], sm[:, C_TC:C_TC + 3],
                                               murstd_c, sm[:, C_TBP:C_TBP + 3],
                                               mul_op, sub_op)
                nc.vector.scalar_tensor_tensor(qkvt[:, 1:3], A_ps[:, 1:3],
                                               rstd_c, cm2[:, 1:3],
                                               mul_op, sub_op)
                qc_t = work.tile([S, 1], fp32, tag="qc_t")
                nc.vector.scalar_tensor_tensor(qc_t[:], A_ps[:, 0:1], rstd_c,
                                               cm2[:, 0:1], mul_op, sub_op)

                # ---- topo attention: q bcast via ones-mat, krs exp scale ----
                Ub_t = work.tile([S, S], bf16, tag="Ub_t")
                nc.vector.tensor_scalar(Ub_t[:], ones_matb[:], qc_t[:],
                                        None, mul_op)
                nc.vector.tensor_copy(pvr_t[:, 0:1], qkvt[:, 2:3])
                nc.tensor.matmul(tsc_ps, Ub_t[:], identb[:],
                                 start=True, stop=True)
                nc.scalar.activation(Et_sb[:], tsc_ps, Exp,
                                     scale=qkvt[:, 1:2])
                nc.tensor.matmul(pvt_ps, Et_sb[:], pvr_t[:],
                                 start=True, stop=True)
                nc.vector.reciprocal(rd1[:], pvt_ps[:, 1:2])
                nc.vector.scalar_tensor_tensor(up_bf[:], pvt_ps[:, 0:1],
                                               rd1[:], u_col[:],
                                               mul_op, add_op)
                nc.vector.scalar_tensor_tensor(up_f[:], pvt_ps[:, 0:1],
                                               rd1[:], u_col[:],
                                               mul_op, add_op)

                # ---- neuron q,k,v columns ----
                for tl in range(TL):
                    nc.tensor.matmul(qkv_ps[:, tl:tl + 1],
                                     tq[:, 3 * tl * S:(3 * tl + 1) * S],
                                     up_bf[:], start=True, stop=True)
                for tl in range(TL):
                    nc.tensor.matmul(qkv_ps[:, 16 + tl:17 + tl],
                                     tq[:, (3 * tl + 1) * S:(3 * tl + 2) * S],
                                     up_bf[:], start=True, stop=True)
                for tl in range(TL):
                    nc.tensor.matmul(qkv_ps[:, 32 + tl:33 + tl],
                                     tq[:, (3 * tl + 2) * S:(3 * tl + 3) * S],
                                     up_bf[:], start=True, stop=True)
                nc.vector.tensor_add(qkb[:], qkv_ps[:, 0:TL],
                                     sm[:, C_QB:C_QB + TL])
                nc.tensor.matmul(tr_ps, qkb[:], identb[:],
                                 start=True, stop=True)
                nc.vector.tensor_copy(qT[:], tr_ps)
                nc.vector.tensor_add(krs_f[:], qkv_ps[:, TL:2 * TL],
                                     sm[:, C_KB:C_KB + TL])
                nc.vector.tensor_mul(vm1[:], qkv_ps[:, 32:48],
                                     sm[:, C_MT:C_MT + TL])
                nc.vector.tensor_add(pvr2[:, :, 0], vm1[:],
                                     sm[:, C_BVMT:C_BVMT + TL])
                nc.vector.tensor_copy(pvr2[:, :, 1], sm[:, C_MT:C_MT + TL])

                # ---- scores: selector q-bcast; krs folded into the exp's
                # per-partition scale; PV per tl ----
                for tl in range(TL):
                    nc.tensor.matmul(sb_slice(tl),
                                     selb[:, tl * S:(tl + 1) * S],
                                     qT[:], start=True, stop=True)
                for tl in range(8):
                    nc.scalar.activation(E_sb[:, tl * S:(tl + 1) * S],
                                         sb_slice(tl),
                                         Exp, scale=krs_f[:, tl:tl + 1])
                for tl in range(8, TL):
                    ts(sc_sb[:, tl * S:(tl + 1) * S],
                       sb_slice(tl),
                       krs_f[:, tl:tl + 1], mul_op)
                for bank in range(2, 4):
                    nc.scalar.activation(E_sb[:, bank * 512:(bank + 1) * 512],
                                         sc_sb[:, bank * 512:(bank + 1) * 512],
                                         Exp)
                for tl in range(TL):
                    nc.tensor.matmul(pvn_ps[:, 2 * tl:2 * tl + 2],
                                     E_sb[:, tl * S:(tl + 1) * S],
                                     pvr[:, 2 * tl:2 * tl + 2],
                                     start=True, stop=True)

                # ---- affine + output ----
                pv2 = pvn_ps.rearrange("p (t k) -> p t k", k=2)
                nc.vector.reciprocal(rden[:], pv2[:, :, 1])
                nc.vector.tensor_mul(zp[:], pv2[:, :, 0], rden[:])
                nc.vector.scalar_tensor_tensor(afr[:], zp[:], up_f[:],
                                               sm[:, C_WMT:C_WMT + TL],
                                               add_op, mul_op)
                nc.tensor.matmul(aff_ps, afr[:], ones_col[:],
                                 start=True, stop=True)
                nc.vector.tensor_add(aff_sb[:], aff_ps,
                                     wbc[:, b:b + 1])

                if b < L - 1:
                    nc.sync.dma_start(ag_in[b][:], aff_sb[:])
                    nc.gpsimd.collective_compute(
                        "AllGather", bypass,
                        replica_groups=[list(range(N_CORES))],
                        ins=[ag_in[b].opt()], outs=[ag_out[b].opt()],
                    )
                    wtiles = load_weights(b + 1, nc.gpsimd)
                else:
                    nc.sync.dma_start(out_d, aff_sb[:])

    nc.compile()
    return nc


def _host_prep(x, W, mask, attn_t, attn_n, norm_params, ada):
    f32 = np.float32
    bf = ml_dtypes.bfloat16
    x, W, mask, attn_t, attn_n, norm_params, ada = (
        np.ascontiguousarray(np.asarray(a, f32))
        for a in (x, W, mask, attn_t, attn_n, norm_params, ada))
    gamma = norm_params[:, 0, :]
    beta = norm_params[:, 1, :]

    rs_vec = np.array([1.0, RS, 1.0], f32)[None, :, None, None]  # scale k rows
    topo_w = attn_t[:, :, :, :S]
    topo_b = attn_t[:, :, :, S]
    topo_wg = topo_w * gamma[:, None, None, :] * rs_vec
    topo_wt_flat = np.ascontiguousarray(
        topo_wg.transpose(0, 3, 1, 2)).reshape(L, S, 3 * S).astype(bf)
    topo_c = topo_wg.sum(axis=3)                                   # (L,3,S)
    topo_bp = (np.einsum('lmis,ls->lmi', topo_w, beta) + topo_b) \
        * rs_vec[:, :, :, 0]

    wmat = W[:, :, :S] * mask
    wbias = W[:, :, S]

    ident = np.eye(S, dtype=bf)
    selb = np.zeros((TL, TL * S), dtype=bf)
    for tl in range(TL):
        selb[tl, tl * S:(tl + 1) * S] = 1.0
    magic = np.array([[MAGIC, 0]], np.int32)
    pre = np.ascontiguousarray(x.reshape(S, 1))

    in_maps = []
    for c in range(N_CORES):
        sl = slice(c * TL, (c + 1) * TL)
        an = attn_n[:, sl]
        anw = an[:, :, :, :, :S] * rs_vec[:, None, :, :, 0, None]  # (L,TL,3,j,i)
        anb = an[:, :, :, :, S] * rs_vec[:, None, :, :, 0]         # (L,TL,3,j)
        tqkv = np.ascontiguousarray(
            anw.transpose(0, 4, 1, 2, 3)).reshape(L, S, TL * 3 * S).astype(bf)

        small = np.zeros((L, S, NC_SM), f32)
        small[:, :, C_QB:C_QB + TL] = anb[:, :, 0, :].transpose(0, 2, 1)
        small[:, :, C_KB:C_KB + TL] = anb[:, :, 1, :].transpose(0, 2, 1)
        small[:, :, C_MT:C_MT + TL] = mask[:, sl].transpose(0, 2, 1)
        small[:, :, C_BVMT:C_BVMT + TL] = (anb[:, :, 2, :]
                                           * mask[:, sl]).transpose(0, 2, 1)
        small[:, :, C_WMT:C_WMT + TL] = wmat[:, sl].transpose(0, 2, 1)
        small[:, :, C_TC:C_TC + 3] = topo_c.transpose(0, 2, 1)
        small[:, :, C_TBP:C_TBP + 3] = topo_bp.transpose(0, 2, 1)
        small[:, :, C_GAM] = gamma
        small[:, :, C_BET] = beta
        small[1:, :, C_G0] = ada[:L - 1, :, 0]
        small[1:, :, C_G1H] = 0.5 * ada[:L - 1, :, 1]

        wbc = np.ascontiguousarray(wbias[:, sl].T)
        in_maps.append(dict(tqkv=tqkv, topo_wt=topo_wt_flat, small=small,
                            pre=pre, wbc=wbc, ident=ident, magic=magic,
                            selb=selb))
    return in_maps


def kernel(x, W, mask, attn_t, attn_n, attn_mask_n, norm_params, ada,
           span_ids, tb_ids):
    global _cached
    from concourse import bass_utils
    if _cached is None:
        _cached = _build()
    nc = _cached
    in_maps = _host_prep(x, W, mask, attn_t, attn_n, norm_params, ada)
    res = bass_utils.run_bass_kernel_spmd(nc, in_maps, core_ids=list(range(N_CORES)))
    out = np.concatenate([res.results[c]["out"].reshape(TL) for c in range(N_CORES)])
    return out.astype(np.float32)
